# revision 1
# baseline (speedup 1.0000x reference)
"""Trainium2 Bass kernel for the MERITS_T patient model (B=1024 data-parallel over 8 cores).

Mathematical simplification of the reference (verified to ~4e-7 rel err in fp32):
  - E_de = _mha(drug_mem, e0, e0) softmaxes over a single key, so its output is
    e0 @ m2_wv @ m2_wo broadcast over all 145 query rows -> the three GATs, the
    graph MHA and drug_mem never reach the output (dead code).
  - e0 = E_en[:, 0] only needs query row 0 of the m1 attention, i.e. only the
    first visit of `med`.
  - patient_j = [glu_rep_j | static]: the static half is visit-independent, so
    it shifts all logits equally (softmax-invariant) and its attention-weighted
    average is just `static` (weights sum to 1). Attention therefore only runs
    on the 32-dim glu half; the static half re-enters linearly at the end via
    SS = sum_h MW_h[32:64] where MW_h = wv_h wo_h m2_wv m2_wo.
  - The gate sigma(x.glu_gate) multiplies logits and values linearly, so it is
    folded in as a scalar after the score reduce / into the softmax weights.
  - final reshape tiles r 145x, so relu(final) @ out_w1 = relu(r) @ sum_m out_w1[m].
    The 43MB sum over m is sharded 8 ways (bf16), partially reduced per core,
    AllGathered (cheaper than AllReduce: no CCE) and tree-summed on device.

All host work is input marshalling only (transpose / reshape / concat / pad /
dtype cast); every arithmetic op runs on device. The dataflow is arranged so
each matmul produces its output pre-transposed for the next consumer; the only
on-device transposes are the four 32-row y_h tiles feeding the final r matmul.
bf16 is used where a single rounding lands well inside the 2e-2 gate: the
out_w1 path, lab path, post-tanh attention tensors, and the final MLP.
"""

import numpy as np
import ml_dtypes

import concourse.bass as bass
import concourse.mybir as mybir
from concourse.bass_utils import run_bass_kernel_spmd
from concourse.tile import TileContext

F32 = mybir.dt.float32
BF16 = mybir.dt.bfloat16
AF = mybir.ActivationFunctionType
ALU = mybir.AluOpType
AX = mybir.AxisListType


def split_multi_waits(nc):
    """The walrus on this image encodes at most ONE sync wait per TPB
    instruction ("Too many sync wait commands" otherwise). Hoist excess waits
    onto standalone InstEventSemaphore ops on the same engine, immediately
    before the instruction — the same mechanism Tile's barriers use."""
    wid = 0
    for f in nc.m.functions:
        for bb in f.blocks:
            out = []
            for ins in bb.instructions:
                si = ins.sync_info
                if si is not None and si.on_wait and len(si.on_wait) > 1:
                    waits = list(si.on_wait)
                    for w in waits[:-1]:
                        wid += 1
                        out.append(mybir.InstEventSemaphore(
                            name=f"Wsplit-{wid}", engine=ins.engine,
                            ins=[], outs=[],
                            sync_info=mybir.SyncInfo(on_wait=[w], on_update=[])))
                    si.on_wait = waits[-1:]
                out.append(ins)
            bb.instructions = out
    return wid


B, T, MED, LAB, GLU, D, H = 1024, 25, 145, 1956, 16, 64, 32
NC_CORES = 8
BC = B // NC_CORES       # 128 patients per core
NH, DH = 4, 16
HID = MED * D // 8       # 1160
MBLK = 20                # 19 real out_w1 blocks per core + 1 zero pad
TP = T + 1               # visit dim padded to 26 (even) for the j-reduce
KLAB = 16                # 2048 = 16*128 lab contraction tiles (1956 + bias + pad)

# column offsets inside the packed small-weight slab [128, PCOLS]
_PC = {}
_o = 0
for _name, _w in [("ident", 128), ("woT", D), ("m2wvT", D), ("m2wo", D),
                  ("mwsb", D), ("mw2sb", D), ("mgT", 1), ("w2sb", H),
                  ("gw3", 2 * H), ("wqT4", NH * D), ("wkT4", NH * D),
                  ("wvT4", NH * D)]:
    _PC[_name] = (_o, _o + _w)
    _o += _w
PCOLS = _o


def build_bass(split_waits=True, debug=False):
    nc = bass.Bass()

    def inp(name, shape, dt=F32):
        return nc.dram_tensor(name, list(shape), dt, kind="ExternalInput")

    dbg_tensors = {}

    def dbg(name, ap):
        if not debug:
            return
        t = nc.dram_tensor("dbg_" + name, list(ap.shape), ap.dtype,
                           kind="ExternalOutput")
        nc.sync.dma_start(out=t[:], in_=ap)
        dbg_tensors[name] = t
    nc.dbg_tensors = dbg_tensors

    # ---- per-core inputs (host-marshalled layouts; see make_in_maps) ----
    w1s_d = inp("w1shard", (128, 580, MBLK), BF16)  # out_w1 shard [(f h), i, m]
    med0T_d = inp("med0T", (MED + 1, BC))           # med visit-0 ^T + ones row
    gluT_d = inp("gluT", (512, BC))                 # glu  [(j f), p] zero-padded
    tfT_d = inp("tfT", (512, BC))                   # time_feat, same layout
    pack_d = inp("packH", (128, PCOLS))             # small weights, packed
    gb8_d = inp("gb8H", (1, 8 * H))                 # glu_b tiled 8x
    ggb_d = inp("ggbH", (1, H), BF16)               # glu_gate
    wbd_d = inp("wbdH", (128, 16 * H))              # block-diag glu_w [glu 8H | tf 8H]
    labT_d = inp("labT", (KLAB * 128, BC), BF16)    # lab^T + ones row + zero pad
    w1sb_d = inp("w1sbH", (128, KLAB * D), BF16)    # sll_w1+b1 as [k, (t d)]
    ow2sb_d = inp("ow2sbH", (128, 10 * MED), BF16)  # out_w2+b2 as [k, (t n)]
    outb1_d = inp("outb1H", (1, HID))
    out_d = nc.dram_tensor("out", [BC, MED], F32, kind="ExternalOutput")

    # internal DRAM for the fp32 W1s AllReduce (CCE adds in fp32 — a bf16
    # collective costs ~7e-3 extra error for only ~3.5us of data phase)
    cc_in = nc.dram_tensor("cc_in", [128, 580], F32)
    cc_out = nc.dram_tensor("cc_out", [128, 580], F32, addr_space="Shared")

    with TileContext(nc) as tc, \
            tc.tile_pool(name="consts", bufs=1) as cp, \
            tc.tile_pool(name="ps", bufs=2, space="PSUM") as ps, \
            tc.tile_pool(name="pst", bufs=1, space="PSUM") as pst, \
            tc.tile_pool(name="psg", bufs=1, space="PSUM") as psg:

        dma = nc.sync.dma_start

        # ================= W1s shard: DMA chunks + reduce + AllReduce ========
        NCH = 8
        bnd = [580 * q // NCH for q in range(NCH + 1)]
        w1raw = cp.tile([128, 580, MBLK], BF16, tag="w1raw")
        ccs = cp.tile([128, 580], F32, tag="ccs")
        for q in range(NCH):
            sl = slice(bnd[q], bnd[q + 1])
            dma(out=w1raw[:, sl, :], in_=w1s_d[:, sl, :])
        for q in range(NCH):
            sl = slice(bnd[q], bnd[q + 1])
            nc.vector.tensor_reduce(out=ccs[:, sl], in_=w1raw[:, sl, :],
                                    axis=AX.X, op=ALU.add)
        dma(out=cc_in[:], in_=ccs)
        nc.gpsimd.collective_compute(
            "AllReduce", ALU.add, replica_groups=[list(range(NC_CORES))],
            ins=[cc_in[:]], outs=[cc_out[:]])
        w1s_sb = cp.tile([D + 1, HID], F32, tag="w1s_sb")
        ccv = cc_out[:].rearrange("(f h) i -> f (h i)", h=2)
        dma(out=w1s_sb[0:D, 0:580], in_=ccv[:, 0:580])
        dma(out=w1s_sb[0:D, 580:HID], in_=ccv[:, 580:HID])
        dma(out=w1s_sb[D:D + 1, :], in_=outb1_d[:])
        dbg("ccs", ccs[:])
        dbg("w1s_sb", w1s_sb[:])

        # ================= bulk input DMAs ===================================
        med0Ta = cp.tile([128, BC], F32, tag="med0Ta")
        dma(out=med0Ta, in_=med0T_d[0:128, :])
        med0Tb = cp.tile([18, BC], F32, tag="med0Tb")
        dma(out=med0Tb, in_=med0T_d[128:MED + 1, :])
        gluT = cp.tile([128, 4, BC], F32, tag="gluT")
        dma(out=gluT, in_=gluT_d[:].rearrange("(c k) p -> k c p", k=128))
        tfT = cp.tile([128, 4, BC], F32, tag="tfT")
        dma(out=tfT, in_=tfT_d[:].rearrange("(c k) p -> k c p", k=128))
        pack = cp.tile([128, PCOLS], F32, tag="pack")
        dma(out=pack, in_=pack_d[:])
        gb8 = cp.tile([1, 8 * H], F32, tag="gb8")
        dma(out=gb8, in_=gb8_d[:])
        wbd = cp.tile([128, 16, H], F32, tag="wbd")
        dma(out=wbd, in_=wbd_d[:].rearrange("k (t h) -> k t h", h=H))
        ggb = cp.tile([128, H], BF16, tag="ggb")
        dma(out=ggb, in_=ggb_d[:].broadcast_to((128, H)))
        labT = cp.tile([128, KLAB, BC], BF16, tag="labT")
        dma(out=labT, in_=labT_d[:].rearrange("(t k) p -> k t p", k=128))
        w1sb = cp.tile([128, KLAB, D], BF16, tag="w1sb")
        dma(out=w1sb, in_=w1sb_d[:].rearrange("k (t d) -> k t d", d=D))
        ow2sb = cp.tile([128, 10, MED], BF16, tag="ow2sb")
        dma(out=ow2sb, in_=ow2sb_d[:].rearrange("k (t n) -> k t n", n=MED))

        def pk(name, rows):
            lo, hi = _PC[name]
            return pack[0:rows, lo:hi]

        ident = pk("ident", 128)
        woT = pk("woT", D)
        m2wvT = pk("m2wvT", D)
        m2wo = pk("m2wo", D)
        mwsb = pk("mwsb", 128)
        mw2sb = pk("mw2sb", 18)
        mgT = pk("mgT", D)
        w2sb = pk("w2sb", D + 1)
        gw3 = pk("gw3", GLU)
        wqT4 = pk("wqT4", DH).rearrange("c (h d) -> c h d", h=NH)
        wkT4 = pk("wkT4", DH).rearrange("c (h d) -> c h d", h=NH)
        wvT4 = pk("wvT4", DH).rearrange("c (h d) -> c h d", h=NH)

        ones1 = cp.tile([1, 128], F32, tag="ones1")
        nc.vector.memset(ones1, 1.0)
        identb = cp.tile([128, 128], BF16, tag="identb")
        nc.vector.tensor_copy(out=identb, in_=ident)

        # ================= weight prep on PE =================================
        # Wvo2 = m2_wv @ m2_wo
        wvo_ps = ps.tile([D, D], F32, tag="acc")
        nc.tensor.matmul(wvo_ps, lhsT=m2wvT, rhs=m2wo)
        wvo2 = cp.tile([D, D], F32, tag="wvo2")
        nc.scalar.copy(out=wvo2, in_=wvo_ps)
        # WoV_h = wo[h-rows] @ Wvo2, all heads -> [c, h, e]
        wov_ps = ps.tile([DH, NH, D], F32, tag="acc")
        for h in range(NH):
            nc.tensor.matmul(wov_ps[:, h, :], lhsT=woT[:, h * DH:(h + 1) * DH],
                             rhs=wvo2[:])
        wov4 = cp.tile([DH, NH, D], F32, tag="wov4")
        nc.scalar.copy(out=wov4, in_=wov_ps)
        # MWg_h = wv_h[0:32 rows] @ WoV_h  -> [f, h, e]; SS = sum_h wv_h[32:] @ WoV_h
        mw_ps = ps.tile([H, NH, D], F32, tag="acc")
        for h in range(NH):
            nc.tensor.matmul(mw_ps[:, h, :], lhsT=wvT4[:, h, 0:H],
                             rhs=wov4[:, h, :])
        mw4 = cp.tile([H, NH, D], F32, tag="mw4")
        nc.scalar.copy(out=mw4, in_=mw_ps)
        ss_ps = ps.tile([H, D], F32, tag="acc")
        for h in range(NH):
            nc.tensor.matmul(ss_ps, lhsT=wvT4[:, h, H:D], rhs=wov4[:, h, :],
                             start=(h == 0), stop=(h == NH - 1))
        ss_sb = cp.tile([H, D], F32, tag="ss_sb")
        nc.scalar.copy(out=ss_sb, in_=ss_ps)
        # A_h[:, 0:32]/4 stacked along free dim -> [d, (h f)]
        ahg_ps = ps.tile([D, NH, H], F32, tag="acc")
        for h in range(NH):
            nc.tensor.matmul(ahg_ps[:, h, :], lhsT=wqT4[:, h, :],
                             rhs=wkT4[:, h, 0:H])
        ahg = cp.tile([D, NH, H], F32, tag="ahg")
        nc.scalar.activation(out=ahg, in_=ahg_ps, func=AF.Copy, scale=1.0 / DH ** 0.5)

        # ================= med visit-0 encoder (all transposed) ==============
        mbTa = cp.tile([128, BC], F32, tag="mbTa")
        nc.vector.tensor_scalar(out=mbTa, in0=med0Ta, scalar1=0.9, scalar2=None,
                                op0=ALU.is_gt)
        mbTb = cp.tile([18, BC], F32, tag="mbTb")
        nc.vector.tensor_scalar(out=mbTb, in0=med0Tb, scalar1=0.9, scalar2=None,
                                op0=ALU.is_gt)
        x0_ps = ps.tile([D, BC], F32, tag="acc")
        nc.tensor.matmul(x0_ps, lhsT=mwsb, rhs=mbTa[:], start=True, stop=False)
        nc.tensor.matmul(x0_ps, lhsT=mw2sb, rhs=mbTb[:], start=False, stop=True)
        x0T = cp.tile([D, BC], F32, tag="x0T")
        nc.vector.tensor_copy(out=x0T, in_=x0_ps)
        g0_ps = ps.tile([1, BC], F32, tag="acc")
        nc.tensor.matmul(g0_ps, lhsT=mgT, rhs=x0T[:])
        sg0T = cp.tile([1, BC], F32, tag="sg0T")
        nc.scalar.activation(out=sg0T, in_=g0_ps, func=AF.Sigmoid)
        sg0r_ps = ps.tile([D, BC], F32, tag="acc")
        nc.tensor.matmul(sg0r_ps, lhsT=ones1[0:1, 0:D], rhs=sg0T[:])
        mr0T = cp.tile([D, BC], F32, tag="mr0T")
        nc.vector.tensor_mul(mr0T, x0T, sg0r_ps)
        # u_g[p, (h f)] = mr0 @ A_h[:, 0:32]
        u_ps = ps.tile([BC, NH, H], F32, tag="acc")
        nc.tensor.matmul(u_ps, lhsT=mr0T[:], rhs=ahg[:].rearrange("d h f -> d (h f)"))
        u_bb = cp.tile([BC, NH, H], BF16, tag="u_bb")
        nc.vector.tensor_copy(out=u_bb, in_=u_ps)

        # ================= glu encoder x = tanh(glu_in @ glu_w + b) ==========
        gx_ps = psg.tile([128, T, H], F32, tag="gx")
        for c in range(3):
            sl8 = slice(8 * c, 8 * c + 8)
            nc.tensor.matmul(gx_ps[:, sl8, :], lhsT=gluT[:, c, :],
                             rhs=wbd[:, 0:8, :], start=True, stop=False)
            nc.tensor.matmul(gx_ps[:, sl8, :], lhsT=tfT[:, c, :],
                             rhs=wbd[:, 8:16, :], start=False, stop=False)
            nc.tensor.matmul(gx_ps[:, sl8, :], lhsT=ones1[0:1, :],
                             rhs=gb8[:], start=False, stop=True)
        nc.tensor.matmul(gx_ps[:, 24, :], lhsT=gluT[0:GLU, 3, :],
                         rhs=gw3[:, 0:H], start=True, stop=False)
        nc.tensor.matmul(gx_ps[:, 24, :], lhsT=tfT[0:GLU, 3, :],
                         rhs=gw3[:, H:2 * H], start=False, stop=False)
        nc.tensor.matmul(gx_ps[:, 24, :], lhsT=ones1[0:1, :],
                         rhs=gb8[0:1, 0:H], start=False, stop=True)
        x_sbb = cp.tile([128, TP, H], BF16, tag="x_sbb")
        nc.vector.memset(x_sbb[:, T, :], 0.0)
        nc.scalar.activation(out=x_sbb[:, 0:T, :], in_=gx_ps, func=AF.Tanh)
        # x transposed to [p, f, j] once, so the value-sum reads contiguously
        xTb = cp.tile([128, H, TP], BF16, tag="xTb")
        nc.vector.tensor_copy(out=xTb, in_=x_sbb.rearrange("p j f -> p f j"))

        # gate = sigmoid(x . glu_gate)
        gm = cp.tile([128, T, H], BF16, tag="gm")
        nc.vector.tensor_mul(gm, x_sbb[:, 0:T, :],
                             ggb[:].unsqueeze(1).broadcast_to((128, T, H)))
        gs = cp.tile([128, T], F32, tag="gs")
        nc.vector.tensor_reduce(out=gs, in_=gm, axis=AX.X, op=ALU.add)
        gate = cp.tile([128, T], F32, tag="gate")
        nc.scalar.activation(out=gate, in_=gs, func=AF.Sigmoid)
        dbg("x_sbb", x_sbb[:])
        dbg("xTb", xTb[:])
        dbg("gate", gate[:])
        dbg("u_bb", u_bb[:])
        dbg("mr0T", mr0T[:])

        # ================= static MLP over lab (all transposed, bf16) ========
        st1_ps = ps.tile([D, BC], F32, tag="acc")
        for t in range(KLAB):
            nc.tensor.matmul(st1_ps, lhsT=w1sb[:, t, :], rhs=labT[:, t, :],
                             start=(t == 0), stop=(t == KLAB - 1))
        st1rT = cp.tile([D + 1, BC], F32, tag="st1rT")
        nc.scalar.activation(out=st1rT[0:D, :], in_=st1_ps, func=AF.Relu)
        nc.vector.memset(st1rT[D:D + 1, :], 1.0)
        st2_ps = ps.tile([H, BC], F32, tag="acc")
        nc.tensor.matmul(st2_ps, lhsT=w2sb, rhs=st1rT[:])
        staticT = cp.tile([H, BC], F32, tag="staticT")
        nc.scalar.activation(out=staticT, in_=st2_ps, func=AF.Relu)

        # ================= one-query attention (glu half only) ===============
        sprod = cp.tile([128, T, NH, H], BF16, tag="sprod")
        nc.vector.tensor_mul(
            sprod,
            x_sbb[:, 0:T, :].unsqueeze(2).broadcast_to((128, T, NH, H)),
            u_bb[:].unsqueeze(1).broadcast_to((128, T, NH, H)))
        s_sb = cp.tile([128, T, NH], BF16, tag="s_sb")
        with nc.allow_low_precision(reason="DVE accumulates fp32 internally; "
                                    "bf16 is a single output rounding"):
            nc.vector.tensor_reduce(out=s_sb, in_=sprod, axis=AX.X, op=ALU.add)
        sg_sb = cp.tile([128, T, NH], F32, tag="sg_sb")
        nc.vector.tensor_mul(sg_sb, s_sb,
                             gate[:].unsqueeze(2).broadcast_to((128, T, NH)))
        es = cp.tile([128, T, NH], F32, tag="es")
        nc.scalar.activation(out=es, in_=sg_sb, func=AF.Exp)
        den = cp.tile([128, NH], F32, tag="den")
        nc.vector.tensor_reduce(out=den, in_=es.rearrange("p j h -> p h j"),
                                axis=AX.X, op=ALU.add)
        rden = cp.tile([128, NH], F32, tag="rden")
        nc.vector.reciprocal(out=rden, in_=den)
        # coef = es * gate * rden, written transposed [p, h, j] (j padded)
        cg = cp.tile([128, T, NH], F32, tag="cg")
        nc.vector.tensor_mul(cg, es, gate[:].unsqueeze(2).broadcast_to((128, T, NH)))
        coefb = cp.tile([128, NH, TP], BF16, tag="coefb")
        nc.vector.memset(coefb[:, :, T], 0.0)
        nc.vector.tensor_mul(
            coefb.rearrange("p h j -> p j h")[:, 0:T, :], cg,
            rden[:].unsqueeze(1).broadcast_to((128, T, NH)))
        # y_glu[p, h, f] = sum_j coef * x  (both operands j-contiguous)
        wprod = cp.tile([128, NH, H, TP], BF16, tag="wprod")
        nc.vector.tensor_mul(
            wprod,
            coefb[:].unsqueeze(2).broadcast_to((128, NH, H, TP)),
            xTb[:].unsqueeze(1).broadcast_to((128, NH, H, TP)))
        y_sbb = cp.tile([128, NH, H], BF16, tag="y_sbb")
        with nc.allow_low_precision(reason="DVE accumulates fp32 internally; "
                                    "bf16 is a single output rounding"):
            nc.vector.tensor_reduce(out=y_sbb, in_=wprod, axis=AX.X, op=ALU.add)

        # rT = sum_h MWg_h^T-free @ y_h^T + SS^T-free @ staticT
        yT4 = cp.tile([H, NH, BC], F32, tag="yT4")
        yt_ps = pst.tile([H, NH, BC], BF16, tag="tp")
        for h in range(NH):
            nc.tensor.transpose(yt_ps[:, h, :], y_sbb[:, h, :], identb[:])
        nc.vector.tensor_copy(out=yT4, in_=yt_ps)
        rT_ps = ps.tile([D, BC], F32, tag="acc")
        for h in range(NH):
            nc.tensor.matmul(rT_ps, lhsT=mw4[:, h, :], rhs=yT4[:, h, :],
                             start=(h == 0), stop=False)
        nc.tensor.matmul(rT_ps, lhsT=ss_sb[:], rhs=staticT[:],
                         start=False, stop=True)
        rrTb = cp.tile([D + 1, BC], F32, tag="rrTb")
        nc.scalar.activation(out=rrTb[0:D, :], in_=rT_ps, func=AF.Relu)
        nc.vector.memset(rrTb[D:D + 1, :], 1.0)
        dbg("s_sb", s_sb[:])
        dbg("coefb", coefb[:])
        dbg("y_sbb", y_sbb[:])
        dbg("staticT", staticT[:])
        dbg("rrTb", rrTb[:])

        # ================= final MLP (transposed tail, bf16) =================
        hidT = cp.tile([128, 10, BC], BF16, tag="hidT")
        # ones everywhere in tile 9; the t=9 relu overwrites rows 0..7 and only
        # rows 0..8 (hid + bias row) feed the final matmul
        nc.vector.memset(hidT[:, 9, :], 1.0)
        for t in range(10):
            n = 128 if t < 9 else 8
            h_ps = ps.tile([128, BC], F32, tag="hacc")
            nc.tensor.matmul(h_ps[0:n, :], lhsT=w1s_sb[:, t * 128:t * 128 + n],
                             rhs=rrTb[:])
            if t % 2 == 0:
                nc.scalar.activation(out=hidT[0:n, t, :], in_=h_ps[0:n, :],
                                     func=AF.Relu)
            else:
                nc.vector.tensor_scalar(out=hidT[0:n, t, :], in0=h_ps[0:n, :],
                                        scalar1=0.0, scalar2=None, op0=ALU.max)
        out_ps = psg.tile([BC, MED], F32, tag="outp")
        for t in range(10):
            k = 128 if t < 9 else 9
            nc.tensor.matmul(out_ps, lhsT=hidT[0:k, t, :], rhs=ow2sb[0:k, t, :],
                             start=(t == 0), stop=(t == 9))
        out_sb = cp.tile([BC, MED], F32, tag="out_sb")
        nc.vector.tensor_copy(out=out_sb, in_=out_ps)
        dma(out=out_d[:], in_=out_sb)

    if split_waits:
        split_multi_waits(nc)
    return nc


_CACHED_NC = None


def make_in_maps(inputs):
    """Pure input marshalling: transpose / reshape / concat / pad / cast only."""
    f = lambda x: np.ascontiguousarray(np.asarray(x, dtype=np.float32))
    cat = np.concatenate
    bf = lambda x: np.ascontiguousarray(np.asarray(x).astype(ml_dtypes.bfloat16))

    lab = f(inputs["lab"])
    glu = f(inputs["glu"]).reshape(B, T * GLU)
    tf = f(inputs["time_feat"]).reshape(B, T * GLU)
    med0 = f(inputs["med"])[:, 0, :]

    # out_w1 -> per-core bf16 shards [(f h), i, m]
    w1b = f(inputs["out_w1"]).reshape(MED, D, HID)
    w1pad = np.zeros((NC_CORES * MBLK, D, HID), np.float32)
    for c in range(NC_CORES):
        lo = c * 19
        n = min(19, MED - lo)
        if n > 0:
            w1pad[c * MBLK:c * MBLK + n] = w1b[lo:lo + n]
    shards = []
    for c in range(NC_CORES):
        blk = w1pad[c * MBLK:(c + 1) * MBLK]               # [20, 64, 1160]
        s = blk.reshape(MBLK, D, 2, 580).transpose(1, 2, 3, 0).reshape(128, 580, MBLK)
        shards.append(bf(s))

    # sll_w1 + bias, padded to 2048 rows, as [k, (t d)] (bf16)
    w1cat = np.zeros((KLAB * 128, D), np.float32)
    w1cat[0:LAB] = f(inputs["sll_w1"])
    w1cat[LAB] = f(inputs["sll_b1"])
    w1sbH = bf(w1cat.reshape(KLAB, 128, D).transpose(1, 0, 2).reshape(128, KLAB * D))

    glu_w = f(inputs["glu_w"])                              # [32, 32]
    gwg, gwt = glu_w[0:GLU], glu_w[GLU:2 * GLU]
    wbdH = np.zeros((128, 16 * H), np.float32)
    for jl in range(8):
        wbdH[jl * GLU:(jl + 1) * GLU, jl * H:(jl + 1) * H] = gwg
        wbdH[jl * GLU:(jl + 1) * GLU, (8 + jl) * H:(9 + jl) * H] = gwt
    gb8H = np.tile(f(inputs["glu_b"]).reshape(1, H), (1, 8))

    medw = f(inputs["med_w"])                               # [145, 64]
    mw2 = cat([medw[128:MED], f(inputs["med_b"]).reshape(1, D)], 0)  # [18, 64]

    def headT(w):  # [64, 64] -> [c, (h d)] with w^T per head block
        wt = f(w).T.reshape(NH, DH, D)                      # [h, c, d]
        return np.ascontiguousarray(wt.transpose(1, 0, 2).reshape(DH, NH * D))

    # packed small-weight slab
    packH = np.zeros((128, PCOLS), np.float32)

    def put(name, arr):
        lo, hi = _PC[name]
        arr = np.asarray(arr, np.float32)
        packH[0:arr.shape[0], lo:hi] = arr

    put("ident", np.eye(128, dtype=np.float32))
    put("woT", f(inputs["m1_wo"]).T)
    put("m2wvT", f(inputs["m2_wv"]).T)
    put("m2wo", f(inputs["m2_wo"]))
    put("mwsb", medw[0:128])
    put("mw2sb", mw2)
    put("mgT", f(inputs["med_gate"]).reshape(D, 1))
    put("w2sb", cat([f(inputs["sll_w2"]), f(inputs["sll_b2"]).reshape(1, H)], 0))
    put("gw3", cat([gwg, gwt], 1))
    put("wqT4", headT(inputs["m1_wq"]))
    put("wkT4", headT(inputs["m1_wk"]))
    put("wvT4", headT(inputs["m1_wv"]))

    # out_w2 + bias, padded to 1280 rows, as [k, (t n)] (bf16)
    w2cat = np.zeros((1280, MED), np.float32)
    w2cat[0:HID] = f(inputs["out_w2"])
    w2cat[HID] = f(inputs["out_b2"])
    ow2sbH = bf(w2cat.reshape(10, 128, MED).transpose(1, 0, 2).reshape(128, 10 * MED))

    rep = {
        "packH": packH, "gb8H": gb8H, "wbdH": wbdH,
        "ggbH": bf(f(inputs["glu_gate"]).reshape(1, H)),
        "w1sbH": w1sbH, "ow2sbH": ow2sbH,
        "outb1H": f(inputs["out_b1"]).reshape(1, HID),
    }

    in_maps = []
    for c in range(NC_CORES):
        sl = slice(c * BC, (c + 1) * BC)
        labTc = np.zeros((KLAB * 128, BC), np.float32)
        labTc[0:LAB] = lab[sl].T
        labTc[LAB] = 1.0
        gluTc = np.zeros((512, BC), np.float32)
        gluTc[0:T * GLU] = glu[sl].T
        tfTc = np.zeros((512, BC), np.float32)
        tfTc[0:T * GLU] = tf[sl].T
        med0Tc = np.ones((MED + 1, BC), np.float32)
        med0Tc[0:MED] = med0[sl].T
        in_maps.append({
            "labT": bf(labTc), "gluT": gluTc, "tfT": tfTc, "med0T": med0Tc,
            "w1shard": shards[c], **rep,
        })
    return in_maps


def kernel(**inputs):
    global _CACHED_NC
    if _CACHED_NC is None:
        _CACHED_NC = build_bass()
    nc = _CACHED_NC
    in_maps = make_in_maps(inputs)
    res = run_bass_kernel_spmd(nc, in_maps, core_ids=list(range(NC_CORES)))
    return np.concatenate([res.results[c]["out"] for c in range(NC_CORES)], axis=0)


if __name__ == "__main__":
    import reference
    inp = reference.setup_inputs()
    out = kernel(**{k: np.asarray(v) for k, v in inp.items()})
    print("kernel output", out.shape, out.dtype)



# revision 4
# speedup vs baseline: 1.2449x; 1.2449x over previous
"""Trainium2 Bass kernel for the MERITS_T patient model (B=1024 data-parallel
over 8 cores), collective-free.

Mathematical simplification of the reference (verified to ~3e-7 rel err fp32):
  - E_de softmaxes over a single key -> GATs / graph-MHA / drug_mem are dead
    code; e0 needs only attention query row 0, i.e. only med[:, 0, :].
  - The static half of patient_j is visit-independent -> softmax-invariant in
    the logits; its attention-weighted value is `static` and re-enters linearly
    via SS = sum_h MW_h[32:64].
  - The gate sigma(x.glu_gate) multiplies logits and values linearly; folded in
    as scalars around the softmax.
  - relu(final) @ out_w1 = relu(r) @ (sum_m out_w1[m]): the 43MB out_w1 only
    enters via its m-block sum W1sum [64, 1160].

Distribution: on this platform ANY firmware collective costs ~90us wall
(model-entry barrier + RDH protocol + launch skew; measured on a trivial
AllGather), and the remote-DMA ISA extension does not compile.  So the kernel
is fully data-parallel with zero cross-core traffic: every core reads the FULL
out_w1 in fp8 (x256, one e4m3 rounding; adds ~4.5e-3 output rel err vs the
2e-2 gate) and reduces the 145 m-blocks itself on the PE via identity-matmul
accumulation into PSUM (m-parity pairs folded into the 128 partitions; the
final MLP contracts both parities at once by duplicating relu(r) rows).
Everything else runs in bf16; sigmoids run as tanh on ScalarE (avoids an
activation-table swap before the Exp).
"""

import numpy as np
import ml_dtypes

import concourse.bass as bass
import concourse.mybir as mybir
from concourse.bass_utils import run_bass_kernel_spmd
from concourse.tile import TileContext

F32 = mybir.dt.float32
BF16 = mybir.dt.bfloat16
FP8 = mybir.dt.float8e4
AF = mybir.ActivationFunctionType
ALU = mybir.AluOpType
AX = mybir.AxisListType


def split_multi_waits(nc):
    """The walrus on this image encodes at most ONE sync wait per TPB
    instruction. Hoist excess waits onto standalone InstEventSemaphore ops."""
    wid = 0
    for f in nc.m.functions:
        for bb in f.blocks:
            out = []
            for ins in bb.instructions:
                si = ins.sync_info
                if si is not None and si.on_wait and len(si.on_wait) > 1:
                    waits = list(si.on_wait)
                    for w in waits[:-1]:
                        wid += 1
                        out.append(mybir.InstEventSemaphore(
                            name=f"Wsplit-{wid}", engine=ins.engine,
                            ins=[], outs=[],
                            sync_info=mybir.SyncInfo(on_wait=[w], on_update=[])))
                    si.on_wait = waits[-1:]
                out.append(ins)
            bb.instructions = out
    return wid


B, T, MED, LAB, GLU, D, H = 1024, 25, 145, 1956, 16, 64, 32
NH, DH = 4, 16
NC_CORES = 8
BC = B // NC_CORES       # 128 patients per core
HID = MED * D // 8       # 1160
KLAB = 16                # 2048 = 16*128 lab contraction tiles
MP = 73                  # m-pairs: 146 m-slots = 145 real + 1 zero pad
TP = T + 1               # 26, padded visit dim for the j-reduce
W1SCALE = 256.0          # fp8 pre-scale (out_w1 sigma~0.01 is subnormal in e4m3)

# column offsets inside the packed small-weight slab [128, PCOLS] (bf16)
_PC = {}
_o = 0
for _name, _w in [("ident", 128), ("woT", D), ("m2wvT", D), ("m2wo", D),
                  ("mwsb", D), ("mw2sb", D), ("mgT", 1), ("w2sb", H),
                  ("gw3", 2 * H), ("wqT4", NH * D), ("wkT4", NH * D),
                  ("wvT4", NH * D), ("wbd", 16 * H)]:
    _PC[_name] = (_o, _o + _w)
    _o += _w
PCOLS = _o


def build_bass():
    nc = bass.Bass()

    def inp(name, shape, dt=F32):
        return nc.dram_tensor(name, list(shape), dt, kind="ExternalInput")

    slab_d = inp("slab", (128, MP * HID), FP8)      # full out_w1, fp8 x256
    labT_d = inp("labT", (KLAB * 128, BC), BF16)    # lab^T + ones row + pad
    gluT_d = inp("gluT", (512, BC), BF16)           # glu [(j f), p] zero-padded
    tfT_d = inp("tfT", (512, BC), BF16)             # time_feat, same layout
    med0T_d = inp("med0T", (MED + 1, BC))           # med visit-0 ^T + ones row
    packB_d = inp("packB", (128, PCOLS), BF16)      # small weights, packed
    rows_d = inp("rowsB", (1, 8 * H + H), BF16)     # glu_b x8 | glu_gate
    outb1_d = inp("outb1T", (128, 10))              # out_b1 as [p, t]
    ow2sb_d = inp("ow2sbH", (128, 10 * MED), BF16)  # out_w2+b2 as [k, (t n)]
    w1sb_d = inp("w1sbH", (128, KLAB * D), BF16)    # sll_w1+b1 as [k, (t d)]
    out_d = nc.dram_tensor("out", [BC, MED], F32, kind="ExternalOutput")

    with TileContext(nc) as tc, \
            tc.tile_pool(name="consts", bufs=1) as cp, \
            tc.tile_pool(name="pa", bufs=3, space="PSUM") as pa, \
            tc.tile_pool(name="ps1", bufs=1, space="PSUM") as ps1, \
            tc.tile_pool(name="pw", bufs=1, space="PSUM") as pw:

        dmaA = nc.scalar.dma_start   # qAct ring: everything but the slab
        dmaS = nc.sync.dma_start     # qSP ring: the big fp8 slab + output

        # ================= input DMAs ====================================
        gluT = cp.tile([128, 4, BC], BF16, tag="gluT")
        dmaA(out=gluT, in_=gluT_d[:].rearrange("(c k) p -> k c p", k=128))
        tfT = cp.tile([128, 4, BC], BF16, tag="tfT")
        dmaA(out=tfT, in_=tfT_d[:].rearrange("(c k) p -> k c p", k=128))
        rows = cp.tile([1, 8 * H + H], BF16, tag="rows")
        dmaA(out=rows, in_=rows_d[:])
        ggb = cp.tile([128, H], BF16, tag="ggb")
        dmaA(out=ggb, in_=rows_d[0:1, 8 * H:8 * H + H].broadcast_to((128, H)))
        pack = cp.tile([128, PCOLS], BF16, tag="pack")
        dmaA(out=pack, in_=packB_d[:])
        med0Ta = cp.tile([128, BC], F32, tag="med0Ta")
        dmaA(out=med0Ta, in_=med0T_d[0:128, :])
        med0Tb = cp.tile([18, BC], F32, tag="med0Tb")
        dmaA(out=med0Tb, in_=med0T_d[128:MED + 1, :])
        w1sb = cp.tile([128, KLAB, D], BF16, tag="w1sb")
        dmaA(out=w1sb, in_=w1sb_d[:].rearrange("k (t d) -> k t d", d=D))
        labT = cp.tile([128, KLAB, BC], BF16, tag="labT")
        dmaA(out=labT, in_=labT_d[:].rearrange("(t k) p -> k t p", k=128))
        outb1T = cp.tile([128, 10], F32, tag="outb1T")
        dmaA(out=outb1T, in_=outb1_d[:])
        ow2sb = cp.tile([128, 10, MED], BF16, tag="ow2sb")
        dmaA(out=ow2sb, in_=ow2sb_d[:].rearrange("k (t n) -> k t n", n=MED))

        slab = cp.tile([128, MP, HID], FP8, tag="slab")
        slab_v = slab_d[:].rearrange("p (m j) -> p m j", j=HID)
        NCH = 16
        bnds = [MP * q // NCH for q in range(NCH + 1)]
        for q in range(NCH):
            dmaS(out=slab[:, bnds[q]:bnds[q + 1], :],
                 in_=slab_v[:, bnds[q]:bnds[q + 1], :])

        def pk(name, nrows):
            lo, hi = _PC[name]
            return pack[0:nrows, lo:hi]

        identb = pk("ident", 128)
        wbd = pk("wbd", 128).rearrange("k (t h) -> k t h", h=H)
        gw3 = pk("gw3", GLU)
        gb8 = rows[0:1, 0:8 * H]

        ones1b = cp.tile([1, 128], BF16, tag="ones1b")
        nc.vector.memset(ones1b, 1.0)
        identF8 = cp.tile([128, 128], FP8, tag="identF8")
        nc.vector.tensor_copy(out=identF8, in_=identb)

        # ================= glu encoder x = tanh(glu_in @ glu_w + b) ======
        x_sbb = cp.tile([128, T, H], BF16, tag="x_sbb")
        xTb = cp.tile([128, H, TP], BF16, tag="xTb")
        nc.vector.memset(xTb[:, :, T:TP], 0.0)
        for c in range(3):
            gx = pa.tile([128, 8, H], F32, tag="pp")
            nc.tensor.matmul(gx, lhsT=gluT[:, c, :], rhs=wbd[:, 0:8, :],
                             start=True, stop=False)
            nc.tensor.matmul(gx, lhsT=tfT[:, c, :], rhs=wbd[:, 8:16, :],
                             start=False, stop=False)
            nc.tensor.matmul(gx, lhsT=ones1b[0:1, :],
                             rhs=gb8.rearrange("a (t h) -> a t h", h=H),
                             start=False, stop=True)
            nc.scalar.activation(out=x_sbb[:, 8 * c:8 * c + 8, :], in_=gx,
                                 func=AF.Tanh)
            nc.scalar.activation(out=xTb[:, :, 8 * c:8 * c + 8],
                                 in_=gx.rearrange("p j f -> p f j"),
                                 func=AF.Tanh)
        gx3 = pa.tile([128, 1, H], F32, tag="pp")
        nc.tensor.matmul(gx3[:, 0, :], lhsT=gluT[0:GLU, 3, :], rhs=gw3[:, 0:H],
                         start=True, stop=False)
        nc.tensor.matmul(gx3[:, 0, :], lhsT=tfT[0:GLU, 3, :], rhs=gw3[:, H:2 * H],
                         start=False, stop=False)
        nc.tensor.matmul(gx3[:, 0, :], lhsT=ones1b[0:1, :], rhs=gb8[0:1, 0:H],
                         start=False, stop=True)
        nc.scalar.activation(out=x_sbb[:, 24:25, :], in_=gx3, func=AF.Tanh)
        nc.scalar.activation(out=xTb[:, :, 24:25],
                             in_=gx3.rearrange("p a f -> p f a"), func=AF.Tanh)

        # ================= weight prep on PE (bf16) ======================
        wvo_ps = pa.tile([D, D], F32, tag="pp")
        nc.tensor.matmul(wvo_ps, lhsT=pk("m2wvT", D), rhs=pk("m2wo", D))
        wvo2b = cp.tile([D, D], BF16, tag="wvo2b")
        nc.scalar.copy(out=wvo2b, in_=wvo_ps)
        woT = pk("woT", D)
        wov_ps = pa.tile([DH, NH, D], F32, tag="pp")
        for h in range(NH):
            nc.tensor.matmul(wov_ps[:, h, :], lhsT=woT[:, h * DH:(h + 1) * DH],
                             rhs=wvo2b[:])
        wov4 = cp.tile([DH, NH, D], BF16, tag="wov4")
        nc.scalar.copy(out=wov4, in_=wov_ps)
        wvT4 = pk("wvT4", DH).rearrange("c (h d) -> c h d", h=NH)
        mw_ps = pa.tile([H, NH, D], F32, tag="pp")
        for h in range(NH):
            nc.tensor.matmul(mw_ps[:, h, :], lhsT=wvT4[:, h, 0:H],
                             rhs=wov4[:, h, :])
        mw4b = cp.tile([H, NH, D], BF16, tag="mw4b")
        nc.scalar.copy(out=mw4b, in_=mw_ps)
        ss_ps = pa.tile([H, D], F32, tag="pp")
        for h in range(NH):
            nc.tensor.matmul(ss_ps, lhsT=wvT4[:, h, H:D], rhs=wov4[:, h, :],
                             start=(h == 0), stop=(h == NH - 1))
        ss_b = cp.tile([H, D], BF16, tag="ss_b")
        nc.scalar.copy(out=ss_b, in_=ss_ps)
        wqT4 = pk("wqT4", DH).rearrange("c (h d) -> c h d", h=NH)
        wkT4 = pk("wkT4", DH).rearrange("c (h d) -> c h d", h=NH)
        ahg_ps = pa.tile([D, NH, H], F32, tag="pp")
        for h in range(NH):
            nc.tensor.matmul(ahg_ps[:, h, :], lhsT=wqT4[:, h, :],
                             rhs=wkT4[:, h, 0:H])
        ahgb = cp.tile([D, NH, H], BF16, tag="ahgb")
        nc.scalar.activation(out=ahgb, in_=ahg_ps, func=AF.Copy,
                             scale=1.0 / DH ** 0.5)

        # ================= med visit-0 encoder (transposed) ==============
        mbTa = cp.tile([128, BC], BF16, tag="mbTa")
        nc.vector.tensor_scalar(out=mbTa, in0=med0Ta, scalar1=0.9, scalar2=None,
                                op0=ALU.is_gt)
        mbTb = cp.tile([18, BC], BF16, tag="mbTb")
        nc.vector.tensor_scalar(out=mbTb, in0=med0Tb, scalar1=0.9, scalar2=None,
                                op0=ALU.is_gt)
        x0_ps = pa.tile([D, BC], F32, tag="pp")
        nc.tensor.matmul(x0_ps, lhsT=pk("mwsb", 128), rhs=mbTa[:],
                         start=True, stop=False)
        nc.tensor.matmul(x0_ps, lhsT=pk("mw2sb", 18), rhs=mbTb[:],
                         start=False, stop=True)
        x0b = cp.tile([D, BC], BF16, tag="x0b")
        nc.vector.tensor_copy(out=x0b, in_=x0_ps)
        g0_ps = pa.tile([1, BC], F32, tag="pp")
        nc.tensor.matmul(g0_ps, lhsT=pk("mgT", D), rhs=x0b[:])
        # sigmoid(z) = 0.5*tanh(z/2) + 0.5 (keeps ScalarE on the tanh table)
        tg0 = cp.tile([1, BC], F32, tag="tg0")
        nc.scalar.activation(out=tg0, in_=g0_ps, func=AF.Tanh, scale=0.5)
        sg0b = cp.tile([1, BC], BF16, tag="sg0b")
        nc.vector.tensor_scalar(out=sg0b, in0=tg0, scalar1=0.5, scalar2=0.5,
                                op0=ALU.mult, op1=ALU.add)
        sg0r_ps = pa.tile([D, BC], F32, tag="pp")
        nc.tensor.matmul(sg0r_ps, lhsT=ones1b[0:1, 0:D], rhs=sg0b[:])
        mr0b = cp.tile([D, BC], BF16, tag="mr0b")
        nc.vector.tensor_mul(mr0b, x0b, sg0r_ps)
        u_ps = pa.tile([BC, NH, H], F32, tag="pp")
        nc.tensor.matmul(u_ps, lhsT=mr0b[:],
                         rhs=ahgb[:].rearrange("d h f -> d (h f)"))
        u_bb = cp.tile([BC, NH, H], BF16, tag="u_bb")
        nc.vector.tensor_copy(out=u_bb, in_=u_ps)

        # ================= gate = sigmoid(x . glu_gate) ==================
        gm = cp.tile([128, T, H], BF16, tag="gm")
        nc.vector.tensor_mul(gm, x_sbb,
                             ggb[:].unsqueeze(1).broadcast_to((128, T, H)))
        gs = cp.tile([128, T], F32, tag="gs")
        nc.vector.tensor_reduce(out=gs, in_=gm, axis=AX.X, op=ALU.add)
        gth = cp.tile([128, T], F32, tag="gth")
        nc.scalar.activation(out=gth, in_=gs, func=AF.Tanh, scale=0.5)
        gate = cp.tile([128, T], F32, tag="gate")
        nc.vector.tensor_scalar(out=gate, in0=gth, scalar1=0.5, scalar2=0.5,
                                op0=ALU.mult, op1=ALU.add)

        # ================= one-query attention (glu half only) ===========
        sprod = cp.tile([128, NH, T, H], BF16, tag="sprod")
        nc.vector.tensor_mul(
            sprod,
            x_sbb[:].unsqueeze(1).broadcast_to((128, NH, T, H)),
            u_bb[:].unsqueeze(2).broadcast_to((128, NH, T, H)))
        s4 = cp.tile([128, NH, T], F32, tag="s4")
        nc.vector.tensor_reduce(out=s4.rearrange("p h j -> p (h j)"),
                                in_=sprod.rearrange("p h j f -> p (h j) f"),
                                axis=AX.X, op=ALU.add)
        sg4 = cp.tile([128, NH, T], F32, tag="sg4")
        nc.vector.tensor_mul(sg4, s4,
                             gate[:].unsqueeze(1).broadcast_to((128, NH, T)))
        es = cp.tile([128, NH, TP], BF16, tag="es")
        nc.vector.memset(es[:, :, T:TP], 0.0)
        nc.scalar.activation(out=es[:, :, 0:T], in_=sg4, func=AF.Exp)
        den = cp.tile([128, NH], F32, tag="den")
        nc.vector.tensor_reduce(out=den, in_=es[:, :, 0:T], axis=AX.X,
                                op=ALU.add)
        rden = cp.tile([128, NH], F32, tag="rden")
        nc.vector.reciprocal(out=rden, in_=den)
        cgb = cp.tile([128, NH, TP], BF16, tag="cgb")
        nc.vector.tensor_mul(cgb[:, :, 0:T], es[:, :, 0:T],
                             gate[:].unsqueeze(1).broadcast_to((128, NH, T)))
        coefb = cp.tile([128, NH, TP], BF16, tag="coefb")
        nc.vector.memset(coefb[:, :, T:TP], 0.0)
        nc.vector.tensor_mul(coefb[:, :, 0:T], cgb[:, :, 0:T],
                             rden[:].unsqueeze(2).broadcast_to((128, NH, T)))
        wprod = cp.tile([128, NH, H, TP], BF16, tag="wprod")
        nc.vector.tensor_mul(
            wprod,
            coefb[:].unsqueeze(2).broadcast_to((128, NH, H, TP)),
            xTb[:].unsqueeze(1).broadcast_to((128, NH, H, TP)))
        y4 = cp.tile([128, NH, H], F32, tag="y4")
        nc.vector.tensor_reduce(out=y4.rearrange("p h f -> p (h f)"),
                                in_=wprod.rearrange("p h f j -> p (h f) j"),
                                axis=AX.X, op=ALU.add)
        y4b = cp.tile([128, NH, H], BF16, tag="y4b")
        nc.vector.tensor_copy(out=y4b, in_=y4)

        # ================= static MLP over lab ===========================
        st1_ps = ps1.tile([D, BC], F32, tag="st1")
        for t in range(KLAB):
            nc.tensor.matmul(st1_ps, lhsT=w1sb[:, t, :], rhs=labT[:, t, :],
                             start=(t == 0), stop=(t == KLAB - 1))
        st1rb = cp.tile([D + 1, BC], BF16, tag="st1rb")
        nc.scalar.activation(out=st1rb[0:D, :], in_=st1_ps, func=AF.Relu)
        nc.vector.memset(st1rb[D:D + 1, :], 1.0)
        st2_ps = pa.tile([H, BC], F32, tag="pp")
        nc.tensor.matmul(st2_ps, lhsT=pk("w2sb", D + 1), rhs=st1rb[:])
        staticb = cp.tile([H, BC], BF16, tag="staticb")
        nc.scalar.activation(out=staticb, in_=st2_ps, func=AF.Relu)

        # ================= W1sum: fp8 ident-matmul reduce over m =========
        # acc[(par f), j] = sum_mp slab[(par f), mp, j]; both m-parities fold
        # later in mm1 via duplicated relu(r) rows.
        acc = pw.tile([128, HID], F32, tag="acc")
        JCH = [(0, 512), (512, 1024), (1024, HID)]

        def slab_block(q):
            for mp in range(bnds[q], bnds[q + 1]):
                for (jl, jh) in JCH:
                    nc.tensor.matmul(acc[:, jl:jh], lhsT=identF8,
                                     rhs=slab[:, mp, jl:jh],
                                     start=(mp == 0), stop=(mp == MP - 1))

        for q in range(12):
            slab_block(q)

        # attention tail interleaved while the last slab chunks stream in
        yt_ps = pa.tile([H, NH, BC], BF16, tag="pp")
        for h in range(NH):
            nc.tensor.transpose(yt_ps[:, h, :], y4b[:, h, :], identb[:])
        yT4b = cp.tile([H, NH, BC], BF16, tag="yT4b")
        nc.scalar.copy(out=yT4b, in_=yt_ps)
        rT_ps = pa.tile([D, BC], F32, tag="pp")
        for h in range(NH):
            nc.tensor.matmul(rT_ps, lhsT=mw4b[:, h, :], rhs=yT4b[:, h, :],
                             start=(h == 0), stop=False)
        nc.tensor.matmul(rT_ps, lhsT=ss_b[:], rhs=staticb[:],
                         start=False, stop=True)
        # relu(r)^T duplicated into both parity halves for the mm1 contraction
        rrT2 = cp.tile([128, BC], BF16, tag="rrT2")
        nc.scalar.activation(out=rrT2[0:D, :], in_=rT_ps, func=AF.Relu)
        nc.scalar.activation(out=rrT2[D:128, :], in_=rT_ps, func=AF.Relu)

        for q in range(12, NCH):
            slab_block(q)
        w1bb = cp.tile([128, HID], BF16, tag="w1bb")
        nc.scalar.copy(out=w1bb, in_=acc)

        # ================= final MLP =====================================
        hidT = cp.tile([128, 10, BC], BF16, tag="hidT")
        nc.vector.memset(hidT[:, 9, :], 1.0)
        for t in range(10):
            n = 128 if t < 9 else 8
            h_ps = pa.tile([128, BC], F32, tag="pp")
            nc.tensor.matmul(h_ps[0:n, :], lhsT=w1bb[:, t * 128:t * 128 + n],
                             rhs=rrT2[:])
            nc.scalar.activation(out=hidT[0:n, t, :], in_=h_ps[0:n, :],
                                 func=AF.Relu, scale=1.0 / W1SCALE,
                                 bias=outb1T[0:n, t:t + 1])
        out_ps = ps1.tile([BC, MED], F32, tag="st1")
        for t in range(10):
            k = 128 if t < 9 else 9
            nc.tensor.matmul(out_ps, lhsT=hidT[0:k, t, :], rhs=ow2sb[0:k, t, :],
                             start=(t == 0), stop=(t == 9))
        out_sb = cp.tile([BC, MED], F32, tag="out_sb")
        nc.vector.tensor_copy(out=out_sb, in_=out_ps)
        dmaS(out=out_d[:], in_=out_sb)

    split_multi_waits(nc)
    return nc


_CACHED_NC = None


def make_in_maps(inputs):
    """Pure input marshalling: transpose / reshape / concat / pad / cast only."""
    f = lambda x: np.ascontiguousarray(np.asarray(x, dtype=np.float32))
    cat = np.concatenate
    bf = lambda x: np.ascontiguousarray(np.asarray(x).astype(ml_dtypes.bfloat16))

    lab = f(inputs["lab"])
    glu = f(inputs["glu"]).reshape(B, T * GLU)
    tf = f(inputs["time_feat"]).reshape(B, T * GLU)
    med0 = f(inputs["med"])[:, 0, :]

    # full out_w1 -> fp8 x256, m-parity folded into partitions: [128, mp, j]
    w1b = f(inputs["out_w1"]).reshape(MED, D, HID)
    w1p = np.zeros((2 * MP, D, HID), np.float32)
    w1p[0:MED] = w1b
    slab = np.ascontiguousarray(
        (w1p.reshape(MP, 2, D, HID).transpose(1, 2, 0, 3)
         .reshape(128, MP * HID) * W1SCALE).astype(ml_dtypes.float8_e4m3))

    # sll_w1 + bias, padded to 2048 rows, as [k, (t d)]
    w1cat = np.zeros((KLAB * 128, D), np.float32)
    w1cat[0:LAB] = f(inputs["sll_w1"])
    w1cat[LAB] = f(inputs["sll_b1"])
    w1sbH = bf(w1cat.reshape(KLAB, 128, D).transpose(1, 0, 2)
               .reshape(128, KLAB * D))

    glu_w = f(inputs["glu_w"])
    gwg, gwt = glu_w[0:GLU], glu_w[GLU:2 * GLU]
    wbdH = np.zeros((128, 16 * H), np.float32)
    for jl in range(8):
        wbdH[jl * GLU:(jl + 1) * GLU, jl * H:(jl + 1) * H] = gwg
        wbdH[jl * GLU:(jl + 1) * GLU, (8 + jl) * H:(9 + jl) * H] = gwt
    gb8H = np.tile(f(inputs["glu_b"]).reshape(1, H), (1, 8))
    rowsB = np.zeros((1, 8 * H + H), np.float32)
    rowsB[0, 0:8 * H] = gb8H
    rowsB[0, 8 * H:] = f(inputs["glu_gate"])

    medw = f(inputs["med_w"])
    mw2 = cat([medw[128:MED], f(inputs["med_b"]).reshape(1, D)], 0)

    def headT(w):  # [64, 64] -> [c, (h d)] with w^T per head block
        wt = f(w).T.reshape(NH, DH, D)
        return np.ascontiguousarray(wt.transpose(1, 0, 2).reshape(DH, NH * D))

    packH = np.zeros((128, PCOLS), np.float32)

    def put(name, arr):
        lo, hi = _PC[name]
        arr = np.asarray(arr, np.float32)
        packH[0:arr.shape[0], lo:hi] = arr

    put("ident", np.eye(128, dtype=np.float32))
    put("woT", f(inputs["m1_wo"]).T)
    put("m2wvT", f(inputs["m2_wv"]).T)
    put("m2wo", f(inputs["m2_wo"]))
    put("mwsb", medw[0:128])
    put("mw2sb", mw2)
    put("mgT", f(inputs["med_gate"]).reshape(D, 1))
    put("w2sb", cat([f(inputs["sll_w2"]), f(inputs["sll_b2"]).reshape(1, H)], 0))
    put("gw3", cat([gwg, gwt], 1))
    put("wqT4", headT(inputs["m1_wq"]))
    put("wkT4", headT(inputs["m1_wk"]))
    put("wvT4", headT(inputs["m1_wv"]))
    put("wbd", wbdH)

    # out_w2 + bias, padded to 1280 rows, as [k, (t n)]
    w2cat = np.zeros((1280, MED), np.float32)
    w2cat[0:HID] = f(inputs["out_w2"])
    w2cat[HID] = f(inputs["out_b2"])
    ow2sbH = bf(w2cat.reshape(10, 128, MED).transpose(1, 0, 2)
                .reshape(128, 10 * MED))

    b1p = np.zeros(1280, np.float32)
    b1p[0:HID] = f(inputs["out_b1"])
    outb1T = np.ascontiguousarray(b1p.reshape(10, 128).T)

    rep = {
        "slab": slab, "packB": bf(packH), "rowsB": bf(rowsB),
        "w1sbH": w1sbH, "ow2sbH": ow2sbH, "outb1T": outb1T,
    }

    in_maps = []
    for c in range(NC_CORES):
        sl = slice(c * BC, (c + 1) * BC)
        labTc = np.zeros((KLAB * 128, BC), np.float32)
        labTc[0:LAB] = lab[sl].T
        labTc[LAB] = 1.0
        gluTc = np.zeros((512, BC), np.float32)
        gluTc[0:T * GLU] = glu[sl].T
        tfTc = np.zeros((512, BC), np.float32)
        tfTc[0:T * GLU] = tf[sl].T
        med0Tc = np.ones((MED + 1, BC), np.float32)
        med0Tc[0:MED] = med0[sl].T
        in_maps.append({
            "labT": bf(labTc), "gluT": bf(gluTc), "tfT": bf(tfTc),
            "med0T": med0Tc, **rep,
        })
    return in_maps


def kernel(**inputs):
    global _CACHED_NC
    if _CACHED_NC is None:
        _CACHED_NC = build_bass()
    nc = _CACHED_NC
    in_maps = make_in_maps(inputs)
    res = run_bass_kernel_spmd(nc, in_maps, core_ids=list(range(NC_CORES)))
    return np.concatenate([res.results[c]["out"] for c in range(NC_CORES)],
                          axis=0)


if __name__ == "__main__":
    import reference
    inp = reference.setup_inputs()
    out = kernel(**{k: np.asarray(v) for k, v in inp.items()})
    print("kernel output", out.shape, out.dtype)


# revision 5
# speedup vs baseline: 1.3543x; 1.0879x over previous
"""Trainium2 Bass kernel for the MERITS_T patient model (B=1024 data-parallel
over 8 cores), collective-free.

Mathematical simplification of the reference (verified to ~3e-7 rel err fp32):
  - E_de softmaxes over a single key -> GATs / graph-MHA / drug_mem are dead
    code; e0 needs only attention query row 0, i.e. only med[:, 0, :].
  - The static half of patient_j is visit-independent -> softmax-invariant in
    the logits; its attention-weighted value is `static` and re-enters linearly
    via SS = sum_h MW_h[32:64].
  - The gate sigma(x.glu_gate) multiplies logits and values linearly; folded in
    as scalars around the softmax.
  - relu(final) @ out_w1 = relu(r) @ (sum_m out_w1[m]): the 43MB out_w1 only
    enters via its m-block sum W1sum [64, 1160].

Distribution: on this platform ANY firmware collective costs ~90us wall
(model-entry barrier + RDH protocol + launch skew; measured on a trivial
AllGather), and the remote-DMA ISA extension does not compile.  So the kernel
is fully data-parallel with zero cross-core traffic: every core reads the FULL
out_w1 in fp8 (x256, one e4m3 rounding; adds ~4.5e-3 output rel err vs the
2e-2 gate) and reduces the 145 m-blocks itself on the PE via identity-matmul
accumulation into PSUM (m-parity pairs folded into the 128 partitions; the
final MLP contracts both parities at once by duplicating relu(r) rows).
Everything else runs in bf16; sigmoids run as tanh on ScalarE (avoids an
activation-table swap before the Exp).
"""

import numpy as np
import ml_dtypes

import concourse.bass as bass
import concourse.mybir as mybir
from concourse.bass_utils import run_bass_kernel_spmd
from concourse.tile import TileContext

F32 = mybir.dt.float32
BF16 = mybir.dt.bfloat16
FP8 = mybir.dt.float8e4
AF = mybir.ActivationFunctionType
ALU = mybir.AluOpType
AX = mybir.AxisListType


def split_multi_waits(nc):
    """The walrus on this image encodes at most ONE sync wait per TPB
    instruction. Hoist excess waits onto standalone InstEventSemaphore ops."""
    wid = 0
    for f in nc.m.functions:
        for bb in f.blocks:
            out = []
            for ins in bb.instructions:
                si = ins.sync_info
                if si is not None and si.on_wait and len(si.on_wait) > 1:
                    waits = list(si.on_wait)
                    for w in waits[:-1]:
                        wid += 1
                        out.append(mybir.InstEventSemaphore(
                            name=f"Wsplit-{wid}", engine=ins.engine,
                            ins=[], outs=[],
                            sync_info=mybir.SyncInfo(on_wait=[w], on_update=[])))
                    si.on_wait = waits[-1:]
                out.append(ins)
            bb.instructions = out
    return wid


B, T, MED, LAB, GLU, D, H = 1024, 25, 145, 1956, 16, 64, 32
NH, DH = 4, 16
NC_CORES = 8
BC = B // NC_CORES       # 128 patients per core
HID = MED * D // 8       # 1160
KLAB = 16                # 2048 = 16*128 lab contraction tiles
MP = 73                  # m-pairs: 146 m-slots = 145 real + 1 zero pad
TP = T + 1               # 26, padded visit dim for the j-reduce
W1SCALE = 256.0          # fp8 pre-scale (out_w1 sigma~0.01 is subnormal in e4m3)

# column offsets inside the packed small-weight slab [128, PCOLS] (bf16)
_PC = {}
_o = 0
for _name, _w in [("ident", 128), ("woT", D), ("m2wvT", D), ("m2wo", D),
                  ("mwsb", D), ("mw2sb", D), ("mgT", 1), ("w2sb", H),
                  ("gw3", 2 * H), ("wqT4", NH * D), ("wkT4", NH * D),
                  ("wvT4", NH * D), ("wbd", 16 * H)]:
    _PC[_name] = (_o, _o + _w)
    _o += _w
PCOLS = _o


def build_bass():
    nc = bass.Bass()

    def inp(name, shape, dt=F32):
        return nc.dram_tensor(name, list(shape), dt, kind="ExternalInput")

    slab_d = inp("slab", (128, MP * HID), FP8)      # full out_w1, fp8 x256
    labT_d = inp("labT", (128, KLAB * BC), BF16)    # lab^T partition-major
    gluT_d = inp("gluT", (128, 4 * BC), BF16)       # glu partition-major
    tfT_d = inp("tfT", (128, 4 * BC), BF16)         # time_feat, same layout
    med0T_d = inp("med0T", (MED + 1, BC))           # med visit-0 ^T + ones row
    packB_d = inp("packB", (128, PCOLS), BF16)      # small weights, packed
    rows_d = inp("rowsB", (1, 8 * H + H), BF16)     # glu_b x8 | glu_gate
    outb1_d = inp("outb1T", (128, 10))              # out_b1 as [p, t]
    ow2sb_d = inp("ow2sbH", (128, 10 * MED), BF16)  # out_w2+b2 as [k, (t n)]
    w1sb_d = inp("w1sbH", (128, KLAB * D), BF16)    # sll_w1+b1 as [k, (t d)]
    out_d = nc.dram_tensor("out", [BC, MED], F32, kind="ExternalOutput")

    with TileContext(nc) as tc, \
            tc.tile_pool(name="consts", bufs=1) as cp, \
            tc.tile_pool(name="pa", bufs=3, space="PSUM") as pa, \
            tc.tile_pool(name="ps1", bufs=1, space="PSUM") as ps1, \
            tc.tile_pool(name="pw", bufs=1, space="PSUM") as pw:

        dmaA = nc.scalar.dma_start   # qAct ring: everything but the slab
        dmaS = nc.sync.dma_start     # qSP ring: the big fp8 slab + output

        # ================= input DMAs ====================================
        # all DRAM layouts are partition-major (host-marshalled), so every
        # transfer is a plain 2D copy with a short descriptor list.
        slab = cp.tile([128, MP, HID], FP8, tag="slab")
        slab_v = slab_d[:].rearrange("p (m j) -> p m j", j=HID)
        NCH = 16
        bnds = [MP * q // NCH for q in range(NCH + 1)]

        def slab_dma(q):
            dmaS(out=slab[:, bnds[q]:bnds[q + 1], :],
                 in_=slab_v[:, bnds[q]:bnds[q + 1], :])

        slab_dma(0)
        gluT = cp.tile([128, 4, BC], BF16, tag="gluT")
        dmaA(out=gluT, in_=gluT_d[:].rearrange("k (c p) -> k c p", p=BC))
        tfT = cp.tile([128, 4, BC], BF16, tag="tfT")
        dmaA(out=tfT, in_=tfT_d[:].rearrange("k (c p) -> k c p", p=BC))
        rows = cp.tile([1, 8 * H + H], BF16, tag="rows")
        dmaA(out=rows, in_=rows_d[:])
        ggb = cp.tile([128, H], BF16, tag="ggb")
        dmaA(out=ggb, in_=rows_d[0:1, 8 * H:8 * H + H].broadcast_to((128, H)))
        pack = cp.tile([128, PCOLS], BF16, tag="pack")
        dmaA(out=pack, in_=packB_d[:])
        med0Ta = cp.tile([128, BC], F32, tag="med0Ta")
        dmaA(out=med0Ta, in_=med0T_d[0:128, :])
        med0Tb = cp.tile([18, BC], F32, tag="med0Tb")
        dmaA(out=med0Tb, in_=med0T_d[128:MED + 1, :])
        for q in range(1, 5):
            slab_dma(q)
        w1sb = cp.tile([128, KLAB, D], BF16, tag="w1sb")
        dmaA(out=w1sb, in_=w1sb_d[:].rearrange("k (t d) -> k t d", d=D))
        labT = cp.tile([128, KLAB, BC], BF16, tag="labT")
        dmaA(out=labT, in_=labT_d[:].rearrange("k (t p) -> k t p", p=BC))
        outb1T = cp.tile([128, 10], F32, tag="outb1T")
        dmaA(out=outb1T, in_=outb1_d[:])
        for q in range(5, NCH):
            slab_dma(q)
        ow2sb = cp.tile([128, 10, MED], BF16, tag="ow2sb")
        dmaA(out=ow2sb, in_=ow2sb_d[:].rearrange("k (t n) -> k t n", n=MED))

        def pk(name, nrows):
            lo, hi = _PC[name]
            return pack[0:nrows, lo:hi]

        identb = pk("ident", 128)
        wbd = pk("wbd", 128).rearrange("k (t h) -> k t h", h=H)
        gw3 = pk("gw3", GLU)
        gb8 = rows[0:1, 0:8 * H]

        ones1b = cp.tile([1, 128], BF16, tag="ones1b")
        nc.vector.memset(ones1b, 1.0)
        identF8 = cp.tile([128, 128], FP8, tag="identF8")
        nc.vector.tensor_copy(out=identF8, in_=identb)

        # ================= glu encoder x = tanh(glu_in @ glu_w + b) ======
        x_sbb = cp.tile([128, T, H], BF16, tag="x_sbb")
        xTb = cp.tile([128, H, TP], BF16, tag="xTb")
        nc.vector.memset(xTb[:, :, T:TP], 0.0)
        for c in range(3):
            gx = pa.tile([128, 8, H], F32, tag="pp")
            nc.tensor.matmul(gx, lhsT=gluT[:, c, :], rhs=wbd[:, 0:8, :],
                             start=True, stop=False)
            nc.tensor.matmul(gx, lhsT=tfT[:, c, :], rhs=wbd[:, 8:16, :],
                             start=False, stop=False)
            nc.tensor.matmul(gx, lhsT=ones1b[0:1, :],
                             rhs=gb8.rearrange("a (t h) -> a t h", h=H),
                             start=False, stop=True)
            nc.scalar.activation(out=x_sbb[:, 8 * c:8 * c + 8, :], in_=gx,
                                 func=AF.Tanh)
            nc.scalar.activation(out=xTb[:, :, 8 * c:8 * c + 8],
                                 in_=gx.rearrange("p j f -> p f j"),
                                 func=AF.Tanh)
        gx3 = pa.tile([128, 1, H], F32, tag="pp")
        nc.tensor.matmul(gx3[:, 0, :], lhsT=gluT[0:GLU, 3, :], rhs=gw3[:, 0:H],
                         start=True, stop=False)
        nc.tensor.matmul(gx3[:, 0, :], lhsT=tfT[0:GLU, 3, :], rhs=gw3[:, H:2 * H],
                         start=False, stop=False)
        nc.tensor.matmul(gx3[:, 0, :], lhsT=ones1b[0:1, :], rhs=gb8[0:1, 0:H],
                         start=False, stop=True)
        nc.scalar.activation(out=x_sbb[:, 24:25, :], in_=gx3, func=AF.Tanh)
        nc.scalar.activation(out=xTb[:, :, 24:25],
                             in_=gx3.rearrange("p a f -> p f a"), func=AF.Tanh)

        # ================= weight prep on PE (bf16) ======================
        wvo_ps = pa.tile([D, D], F32, tag="pp")
        nc.tensor.matmul(wvo_ps, lhsT=pk("m2wvT", D), rhs=pk("m2wo", D))
        wvo2b = cp.tile([D, D], BF16, tag="wvo2b")
        nc.scalar.copy(out=wvo2b, in_=wvo_ps)
        woT = pk("woT", D)
        wov_ps = pa.tile([DH, NH, D], F32, tag="pp")
        for h in range(NH):
            nc.tensor.matmul(wov_ps[:, h, :], lhsT=woT[:, h * DH:(h + 1) * DH],
                             rhs=wvo2b[:])
        wov4 = cp.tile([DH, NH, D], BF16, tag="wov4")
        nc.scalar.copy(out=wov4, in_=wov_ps)
        wvT4 = pk("wvT4", DH).rearrange("c (h d) -> c h d", h=NH)
        mw_ps = pa.tile([H, NH, D], F32, tag="pp")
        for h in range(NH):
            nc.tensor.matmul(mw_ps[:, h, :], lhsT=wvT4[:, h, 0:H],
                             rhs=wov4[:, h, :])
        mw4b = cp.tile([H, NH, D], BF16, tag="mw4b")
        nc.scalar.copy(out=mw4b, in_=mw_ps)
        ss_ps = pa.tile([H, D], F32, tag="pp")
        for h in range(NH):
            nc.tensor.matmul(ss_ps, lhsT=wvT4[:, h, H:D], rhs=wov4[:, h, :],
                             start=(h == 0), stop=(h == NH - 1))
        ss_b = cp.tile([H, D], BF16, tag="ss_b")
        nc.scalar.copy(out=ss_b, in_=ss_ps)
        wqT4 = pk("wqT4", DH).rearrange("c (h d) -> c h d", h=NH)
        wkT4 = pk("wkT4", DH).rearrange("c (h d) -> c h d", h=NH)
        ahg_ps = pa.tile([D, NH, H], F32, tag="pp")
        for h in range(NH):
            nc.tensor.matmul(ahg_ps[:, h, :], lhsT=wqT4[:, h, :],
                             rhs=wkT4[:, h, 0:H])
        ahgb = cp.tile([D, NH, H], BF16, tag="ahgb")
        nc.scalar.activation(out=ahgb, in_=ahg_ps, func=AF.Copy,
                             scale=1.0 / DH ** 0.5)

        # ================= med visit-0 encoder (transposed) ==============
        mbTa = cp.tile([128, BC], BF16, tag="mbTa")
        nc.vector.tensor_scalar(out=mbTa, in0=med0Ta, scalar1=0.9, scalar2=None,
                                op0=ALU.is_gt)
        mbTb = cp.tile([18, BC], BF16, tag="mbTb")
        nc.vector.tensor_scalar(out=mbTb, in0=med0Tb, scalar1=0.9, scalar2=None,
                                op0=ALU.is_gt)
        x0_ps = pa.tile([D, BC], F32, tag="pp")
        nc.tensor.matmul(x0_ps, lhsT=pk("mwsb", 128), rhs=mbTa[:],
                         start=True, stop=False)
        nc.tensor.matmul(x0_ps, lhsT=pk("mw2sb", 18), rhs=mbTb[:],
                         start=False, stop=True)
        x0b = cp.tile([D, BC], BF16, tag="x0b")
        nc.vector.tensor_copy(out=x0b, in_=x0_ps)
        g0_ps = pa.tile([1, BC], F32, tag="pp")
        nc.tensor.matmul(g0_ps, lhsT=pk("mgT", D), rhs=x0b[:])
        # sigmoid(z) = 0.5*tanh(z/2) + 0.5 (keeps ScalarE on the tanh table)
        tg0 = cp.tile([1, BC], F32, tag="tg0")
        nc.scalar.activation(out=tg0, in_=g0_ps, func=AF.Tanh, scale=0.5)
        sg0b = cp.tile([1, BC], BF16, tag="sg0b")
        nc.vector.tensor_scalar(out=sg0b, in0=tg0, scalar1=0.5, scalar2=0.5,
                                op0=ALU.mult, op1=ALU.add)
        sg0r_ps = pa.tile([D, BC], F32, tag="pp")
        nc.tensor.matmul(sg0r_ps, lhsT=ones1b[0:1, 0:D], rhs=sg0b[:])
        mr0b = cp.tile([D, BC], BF16, tag="mr0b")
        nc.vector.tensor_mul(mr0b, x0b, sg0r_ps)
        u_ps = pa.tile([BC, NH, H], F32, tag="pp")
        nc.tensor.matmul(u_ps, lhsT=mr0b[:],
                         rhs=ahgb[:].rearrange("d h f -> d (h f)"))
        u_bb = cp.tile([BC, NH, H], BF16, tag="u_bb")
        nc.vector.tensor_copy(out=u_bb, in_=u_ps)

        # ================= gate = sigmoid(x . glu_gate) ==================
        gm = cp.tile([128, T, H], BF16, tag="gm")
        nc.vector.tensor_mul(gm, x_sbb,
                             ggb[:].unsqueeze(1).broadcast_to((128, T, H)))
        gs = cp.tile([128, T], F32, tag="gs")
        nc.vector.tensor_reduce(out=gs, in_=gm, axis=AX.X, op=ALU.add)
        gth = cp.tile([128, T], F32, tag="gth")
        nc.scalar.activation(out=gth, in_=gs, func=AF.Tanh, scale=0.5)
        gate = cp.tile([128, T], F32, tag="gate")
        nc.vector.tensor_scalar(out=gate, in0=gth, scalar1=0.5, scalar2=0.5,
                                op0=ALU.mult, op1=ALU.add)

        # ================= one-query attention (glu half only) ===========
        sprod = cp.tile([128, NH, T, H], BF16, tag="sprod")
        nc.vector.tensor_mul(
            sprod,
            x_sbb[:].unsqueeze(1).broadcast_to((128, NH, T, H)),
            u_bb[:].unsqueeze(2).broadcast_to((128, NH, T, H)))
        s4 = cp.tile([128, NH, T], F32, tag="s4")
        nc.vector.tensor_reduce(out=s4.rearrange("p h j -> p (h j)"),
                                in_=sprod.rearrange("p h j f -> p (h j) f"),
                                axis=AX.X, op=ALU.add)
        sg4 = cp.tile([128, NH, T], F32, tag="sg4")
        nc.vector.tensor_mul(sg4, s4,
                             gate[:].unsqueeze(1).broadcast_to((128, NH, T)))
        es = cp.tile([128, NH, TP], BF16, tag="es")
        nc.vector.memset(es[:, :, T:TP], 0.0)
        nc.scalar.activation(out=es[:, :, 0:T], in_=sg4, func=AF.Exp)
        den = cp.tile([128, NH], F32, tag="den")
        nc.vector.tensor_reduce(out=den, in_=es[:, :, 0:T], axis=AX.X,
                                op=ALU.add)
        rden = cp.tile([128, NH], F32, tag="rden")
        nc.vector.reciprocal(out=rden, in_=den)
        cgb = cp.tile([128, NH, TP], BF16, tag="cgb")
        nc.vector.tensor_mul(cgb[:, :, 0:T], es[:, :, 0:T],
                             gate[:].unsqueeze(1).broadcast_to((128, NH, T)))
        coefb = cp.tile([128, NH, TP], BF16, tag="coefb")
        nc.vector.memset(coefb[:, :, T:TP], 0.0)
        nc.vector.tensor_mul(coefb[:, :, 0:T], cgb[:, :, 0:T],
                             rden[:].unsqueeze(2).broadcast_to((128, NH, T)))
        wprod = cp.tile([128, NH, H, TP], BF16, tag="wprod")
        nc.vector.tensor_mul(
            wprod,
            coefb[:].unsqueeze(2).broadcast_to((128, NH, H, TP)),
            xTb[:].unsqueeze(1).broadcast_to((128, NH, H, TP)))
        y4 = cp.tile([128, NH, H], F32, tag="y4")
        nc.vector.tensor_reduce(out=y4.rearrange("p h f -> p (h f)"),
                                in_=wprod.rearrange("p h f j -> p (h f) j"),
                                axis=AX.X, op=ALU.add)
        y4b = cp.tile([128, NH, H], BF16, tag="y4b")
        nc.vector.tensor_copy(out=y4b, in_=y4)

        # ================= static MLP over lab ===========================
        st1_ps = ps1.tile([D, BC], F32, tag="st1")
        for t in range(KLAB):
            nc.tensor.matmul(st1_ps, lhsT=w1sb[:, t, :], rhs=labT[:, t, :],
                             start=(t == 0), stop=(t == KLAB - 1))
        st1rb = cp.tile([D + 1, BC], BF16, tag="st1rb")
        nc.scalar.activation(out=st1rb[0:D, :], in_=st1_ps, func=AF.Relu)
        nc.vector.memset(st1rb[D:D + 1, :], 1.0)
        st2_ps = pa.tile([H, BC], F32, tag="pp")
        nc.tensor.matmul(st2_ps, lhsT=pk("w2sb", D + 1), rhs=st1rb[:])
        staticb = cp.tile([H, BC], BF16, tag="staticb")
        nc.scalar.activation(out=staticb, in_=st2_ps, func=AF.Relu)

        # ================= W1sum: fp8 ident-matmul reduce over m =========
        # acc[(par f), j] = sum_mp slab[(par f), mp, j]; both m-parities fold
        # later in mm1 via duplicated relu(r) rows.
        acc = pw.tile([128, HID], F32, tag="acc")
        JCH = [(0, 512), (512, 1024), (1024, HID)]

        def slab_block(q):
            for mp in range(bnds[q], bnds[q + 1]):
                for (jl, jh) in JCH:
                    nc.tensor.matmul(acc[:, jl:jh], lhsT=identF8,
                                     rhs=slab[:, mp, jl:jh],
                                     start=(mp == 0), stop=(mp == MP - 1))

        for q in range(12):
            slab_block(q)

        # attention tail interleaved while the last slab chunks stream in
        yt_ps = pa.tile([H, NH, BC], BF16, tag="pp")
        for h in range(NH):
            nc.tensor.transpose(yt_ps[:, h, :], y4b[:, h, :], identb[:])
        yT4b = cp.tile([H, NH, BC], BF16, tag="yT4b")
        nc.scalar.copy(out=yT4b, in_=yt_ps)
        rT_ps = pa.tile([D, BC], F32, tag="pp")
        for h in range(NH):
            nc.tensor.matmul(rT_ps, lhsT=mw4b[:, h, :], rhs=yT4b[:, h, :],
                             start=(h == 0), stop=False)
        nc.tensor.matmul(rT_ps, lhsT=ss_b[:], rhs=staticb[:],
                         start=False, stop=True)
        # relu(r)^T duplicated into both parity halves for the mm1 contraction
        rrT2 = cp.tile([128, BC], BF16, tag="rrT2")
        nc.scalar.activation(out=rrT2[0:D, :], in_=rT_ps, func=AF.Relu)
        nc.scalar.activation(out=rrT2[D:128, :], in_=rT_ps, func=AF.Relu)

        for q in range(12, NCH):
            slab_block(q)
        w1bb = cp.tile([128, HID], BF16, tag="w1bb")
        nc.scalar.copy(out=w1bb, in_=acc)

        # ================= final MLP =====================================
        hidT = cp.tile([128, 10, BC], BF16, tag="hidT")
        nc.vector.memset(hidT[:, 9, :], 1.0)
        for t in range(10):
            n = 128 if t < 9 else 8
            h_ps = pa.tile([128, BC], F32, tag="pp")
            nc.tensor.matmul(h_ps[0:n, :], lhsT=w1bb[:, t * 128:t * 128 + n],
                             rhs=rrT2[:])
            nc.scalar.activation(out=hidT[0:n, t, :], in_=h_ps[0:n, :],
                                 func=AF.Relu, scale=1.0 / W1SCALE,
                                 bias=outb1T[0:n, t:t + 1])
        out_ps = ps1.tile([BC, MED], F32, tag="st1")
        for t in range(10):
            k = 128 if t < 9 else 9
            nc.tensor.matmul(out_ps, lhsT=hidT[0:k, t, :], rhs=ow2sb[0:k, t, :],
                             start=(t == 0), stop=(t == 9))
        out_sb = cp.tile([BC, MED], F32, tag="out_sb")
        nc.vector.tensor_copy(out=out_sb, in_=out_ps)
        dmaS(out=out_d[:], in_=out_sb)

    split_multi_waits(nc)
    return nc


_CACHED_NC = None


def make_in_maps(inputs):
    """Pure input marshalling: transpose / reshape / concat / pad / cast only."""
    f = lambda x: np.ascontiguousarray(np.asarray(x, dtype=np.float32))
    cat = np.concatenate
    bf = lambda x: np.ascontiguousarray(np.asarray(x).astype(ml_dtypes.bfloat16))

    lab = f(inputs["lab"])
    glu = f(inputs["glu"]).reshape(B, T * GLU)
    tf = f(inputs["time_feat"]).reshape(B, T * GLU)
    med0 = f(inputs["med"])[:, 0, :]

    # full out_w1 -> fp8 x256, m-parity folded into partitions: [128, mp, j]
    w1b = f(inputs["out_w1"]).reshape(MED, D, HID)
    w1p = np.zeros((2 * MP, D, HID), np.float32)
    w1p[0:MED] = w1b
    slab = np.ascontiguousarray(
        (w1p.reshape(MP, 2, D, HID).transpose(1, 2, 0, 3)
         .reshape(128, MP * HID) * W1SCALE).astype(ml_dtypes.float8_e4m3))

    # sll_w1 + bias, padded to 2048 rows, as [k, (t d)]
    w1cat = np.zeros((KLAB * 128, D), np.float32)
    w1cat[0:LAB] = f(inputs["sll_w1"])
    w1cat[LAB] = f(inputs["sll_b1"])
    w1sbH = bf(w1cat.reshape(KLAB, 128, D).transpose(1, 0, 2)
               .reshape(128, KLAB * D))

    glu_w = f(inputs["glu_w"])
    gwg, gwt = glu_w[0:GLU], glu_w[GLU:2 * GLU]
    wbdH = np.zeros((128, 16 * H), np.float32)
    for jl in range(8):
        wbdH[jl * GLU:(jl + 1) * GLU, jl * H:(jl + 1) * H] = gwg
        wbdH[jl * GLU:(jl + 1) * GLU, (8 + jl) * H:(9 + jl) * H] = gwt
    gb8H = np.tile(f(inputs["glu_b"]).reshape(1, H), (1, 8))
    rowsB = np.zeros((1, 8 * H + H), np.float32)
    rowsB[0, 0:8 * H] = gb8H
    rowsB[0, 8 * H:] = f(inputs["glu_gate"])

    medw = f(inputs["med_w"])
    mw2 = cat([medw[128:MED], f(inputs["med_b"]).reshape(1, D)], 0)

    def headT(w):  # [64, 64] -> [c, (h d)] with w^T per head block
        wt = f(w).T.reshape(NH, DH, D)
        return np.ascontiguousarray(wt.transpose(1, 0, 2).reshape(DH, NH * D))

    packH = np.zeros((128, PCOLS), np.float32)

    def put(name, arr):
        lo, hi = _PC[name]
        arr = np.asarray(arr, np.float32)
        packH[0:arr.shape[0], lo:hi] = arr

    put("ident", np.eye(128, dtype=np.float32))
    put("woT", f(inputs["m1_wo"]).T)
    put("m2wvT", f(inputs["m2_wv"]).T)
    put("m2wo", f(inputs["m2_wo"]))
    put("mwsb", medw[0:128])
    put("mw2sb", mw2)
    put("mgT", f(inputs["med_gate"]).reshape(D, 1))
    put("w2sb", cat([f(inputs["sll_w2"]), f(inputs["sll_b2"]).reshape(1, H)], 0))
    put("gw3", cat([gwg, gwt], 1))
    put("wqT4", headT(inputs["m1_wq"]))
    put("wkT4", headT(inputs["m1_wk"]))
    put("wvT4", headT(inputs["m1_wv"]))
    put("wbd", wbdH)

    # out_w2 + bias, padded to 1280 rows, as [k, (t n)]
    w2cat = np.zeros((1280, MED), np.float32)
    w2cat[0:HID] = f(inputs["out_w2"])
    w2cat[HID] = f(inputs["out_b2"])
    ow2sbH = bf(w2cat.reshape(10, 128, MED).transpose(1, 0, 2)
                .reshape(128, 10 * MED))

    b1p = np.zeros(1280, np.float32)
    b1p[0:HID] = f(inputs["out_b1"])
    outb1T = np.ascontiguousarray(b1p.reshape(10, 128).T)

    rep = {
        "slab": slab, "packB": bf(packH), "rowsB": bf(rowsB),
        "w1sbH": w1sbH, "ow2sbH": ow2sbH, "outb1T": outb1T,
    }

    in_maps = []
    for c in range(NC_CORES):
        sl = slice(c * BC, (c + 1) * BC)
        labTc = np.zeros((KLAB * 128, BC), np.float32)
        labTc[0:LAB] = lab[sl].T
        labTc[LAB] = 1.0
        labTc = labTc.reshape(KLAB, 128, BC).transpose(1, 0, 2).reshape(128, KLAB * BC)
        gluTc = np.zeros((512, BC), np.float32)
        gluTc[0:T * GLU] = glu[sl].T
        gluTc = gluTc.reshape(4, 128, BC).transpose(1, 0, 2).reshape(128, 4 * BC)
        tfTc = np.zeros((512, BC), np.float32)
        tfTc[0:T * GLU] = tf[sl].T
        tfTc = tfTc.reshape(4, 128, BC).transpose(1, 0, 2).reshape(128, 4 * BC)
        med0Tc = np.ones((MED + 1, BC), np.float32)
        med0Tc[0:MED] = med0[sl].T
        in_maps.append({
            "labT": bf(labTc), "gluT": bf(gluTc), "tfT": bf(tfTc),
            "med0T": med0Tc, **rep,
        })
    return in_maps


def kernel(**inputs):
    global _CACHED_NC
    if _CACHED_NC is None:
        _CACHED_NC = build_bass()
    nc = _CACHED_NC
    in_maps = make_in_maps(inputs)
    res = run_bass_kernel_spmd(nc, in_maps, core_ids=list(range(NC_CORES)))
    return np.concatenate([res.results[c]["out"] for c in range(NC_CORES)],
                          axis=0)


if __name__ == "__main__":
    import reference
    inp = reference.setup_inputs()
    out = kernel(**{k: np.asarray(v) for k, v in inp.items()})
    print("kernel output", out.shape, out.dtype)
